# revision 3
# baseline (speedup 1.0000x reference)
"""nn_CoupFourGAT — Trainium2 Bass kernel.

Host (numpy): STFT framing + rFFT + map_w projection + 3x3 conv + QKV
projections + gate-fusion MLP + decoder (all tiny, ~100 MFLOP total).
Device (8 NeuronCores, SPMD): the dominant masked-softmax GAT attention:
per (batch, re/im, head) triple computes
  E^T = adj^T * exp(scale * K Q^T);  raw = [E @ V | E @ 1]
with the softmax denominator fused in as an extra ones-column of V.
Data-parallel over batch: core c handles batches [16c, 16c+16).

v2 design (ACT-saturated pipeline):
 - MM1 (K Q^T) in fp8e4m3 DoubleRow mode: contraction l=12 packed as
   [6 partitions x 2 k-tiles]; per (triple, m-chunk) one matmul of 247
   moving cols at 0.5 cyc/row. 3 triples per 32-partition group.
 - scores psum tile [128, 4, 256] f32 (2 banks) holds 2 triples x 2
   m-chunks (m 0:128 / 119:247, 247 query cols each).
 - one exp ACT per pair ([128, 4x247] strided AP, scale folded in,
   bf16 out) -> one DVE adj-mask mul (bf16 2x mode).
 - MM2 (E^T.T @ Vp, em-stationary) accumulates 8 triples per PSUM
   bank; DVE drains PSUM->SBUF; 8 output DMAs.
 - psum: scores pool bufs=3 (6 banks) + po pool bufs=2 (2 banks);
   MM1 emitted 2 iterations ahead of MM2 so the in-order PE queue
   never blocks the next ACT.
"""
import math
import os
import numpy as np
import ml_dtypes

B, N, L, H = 128, 247, 12, 4
NFFT, HOP, FRAMES = 256, 246, 13
LAM = 0.01
NCORES = 8
BLOC = B // NCORES          # 16 batches per core
NTRI = BLOC * 2 * H         # 128 (batch, re/im, head) triples per core
NBIG = 4                    # output big groups per core
TPB = NTRI // NBIG          # 32 triples per output big group
NSLOT = (NTRI + 2) // 3     # 43 slots of 3 triples (bases {0,32,64})
SPT = 8                     # slots per qt/ka/kb tile
NQKT = (NSLOT + SPT - 1) // SPT  # 6 q/k tiles
NU = NTRI // 2              # 64 u-iterations (2 triples each)
KP, KT = 6, 2               # DoubleRow packing: l = KP*jj + p
SCALE = 1.0 / math.sqrt(L)
BF16 = ml_dtypes.bfloat16
FP8 = ml_dtypes.float8_e4m3

_DEVICE_CACHE = {}
LAST_EXEC_NS = None


def _erf(x):
    try:
        from scipy.special import erf
        return erf(x)
    except Exception:
        return np.vectorize(math.erf, otypes=[np.float64])(x)


def _ln(t, g, b, eps=1e-5):
    m = t.mean(-1, keepdims=True)
    v = ((t - m) ** 2).mean(-1, keepdims=True)
    return (t - m) / np.sqrt(v + eps) * g + b


def _softshrink(t):
    return np.where(t > LAM, t - LAM, np.where(t < -LAM, t + LAM, 0.0))


def _leaky(t):
    return np.where(t >= 0, t, 0.01 * t)


def _front(x, map_w, map_b, conv_w, conv_b):
    """x (B,N,L) -> tr, ti (B,N,L): FFT + map + conv + leaky + residual."""
    Bc = x.shape[0]
    xf = x.reshape(Bc, -1)
    pad = np.pad(xf, ((0, 0), (NFFT // 2, NFFT // 2)), mode='reflect')
    idx = np.arange(FRAMES)[:, None] * HOP + np.arange(NFFT)[None, :]
    frames = pad[:, idx]                               # (B, 13, 256)
    spec = np.fft.rfft(frames.astype(np.float64), axis=-1)  # (B,13,129)
    spec = np.swapaxes(spec, 1, 2)[:, :, :L]           # (B, 129, 12)
    xr = spec.real.reshape(Bc, L, 129) @ map_w.T.astype(np.float64) + map_b
    xi = spec.imag.reshape(Bc, L, 129) @ map_w.T.astype(np.float64) + map_b
    xr = xr.reshape(Bc, N, L).astype(np.float32)
    xi = xi.reshape(Bc, N, L).astype(np.float32)
    vec = np.stack([xr, xi], axis=-1)                  # (B, N, L, 2)
    v2 = vec.reshape(Bc, 2, N, L)
    vp = np.pad(v2, ((0, 0), (0, 0), (1, 1), (1, 1)))
    c = np.zeros_like(v2)
    for o in range(2):
        for i in range(2):
            for ky in range(3):
                for kx in range(3):
                    c[:, o] += conv_w[o, i, ky, kx] * vp[:, i, ky:ky + N, kx:kx + L]
    c = c + conv_b[None, :, None, None]
    c = _leaky(c.reshape(Bc, N, L, 2)) + vec
    return c[..., 0], c[..., 1]


def _pack_host(qt, kt, vp, adj):
    """qt/kt: (NC, NTRI, L, N) f32; vp: (NC, NTRI, N, 13) f32; adj (N, N).

    Returns per-core device arrays:
      qt8 (NC, NQKT, 96, 2, SPT, 256) fp8   [l = 6*jj + p, query col n]
      ka8/kb8 (NC, NQKT, 96, 2, SPT, 128)   [key rows m 0:128 / 119:247]
      vpa/vpb (NC, 128, NTRI, 13) bf16      [m-chunks, overlap zeroed in B]
      adjm (128, 4, 247) bf16               [mask blocks A,B,A,B]
    """
    NC = NCORES

    def qk_pack(a, width, mlo):
        # a: (NC, NTRI, L, >=mlo+width):
        # out[c, ti, 32j+p, jj, s, w] = a[c, t=3(8ti+s)+j, l=6jj+p, mlo+w]
        out = np.zeros((NC, NQKT, 3, 32, KT, SPT, width), FP8)
        pad = np.zeros((NC, NQKT * SPT * 3, L, width), a.dtype)
        pad[:, :NTRI] = a[:, :, :, mlo:mlo + width]
        src = pad.reshape(NC, NQKT, SPT, 3, KT, KP, width)
        out[:, :, :, :KP] = src.transpose(0, 1, 3, 5, 4, 2, 6).astype(FP8)
        return np.ascontiguousarray(
            out.reshape(NC, NQKT, 96, KT, SPT, width))

    qn = np.zeros((NC, NTRI, L, 256), np.float32)
    qn[:, :, :, :N] = qt
    qt8 = qk_pack(qn, 256, 0)
    ka8 = qk_pack(kt, 128, 0)
    kb8 = qk_pack(kt, 128, 119)

    vpa = vp[:, :, :128, :]                                  # (NC,NTRI,128,13)
    vpb = vp[:, :, 119:, :].copy()
    vpb[:, :, :9, :] = 0.0                                   # kill overlap
    vpa_h = np.ascontiguousarray(vpa.transpose(0, 2, 1, 3)).astype(BF16)
    vpb_h = np.ascontiguousarray(vpb.transpose(0, 2, 1, 3)).astype(BF16)

    adjT = adj.T.astype(np.float32)                          # adjT[m,n]
    mA = adjT[:128, :]                                       # (128, 247)
    mB = adjT[119:, :]
    adjm_h = np.ascontiguousarray(
        np.stack([mA, mB, mA, mB], axis=1)).astype(BF16)     # (128, 4, 247)
    return qt8, ka8, kb8, vpa_h, vpb_h, adjm_h


def _unpack_raw(rawA, rawB):
    """rawA/rawB (NC, NBIG, 128, 416) f32 -> raw (NC, NTRI, N, 13)."""
    NC = NCORES
    rA = rawA.reshape(NC, NBIG, 128, TPB, 13).transpose(0, 1, 3, 2, 4)
    rA = rA.reshape(NC, NTRI, 128, 13)
    rB = rawB.reshape(NC, NBIG, 128, TPB, 13).transpose(0, 1, 3, 2, 4)
    rB = rB.reshape(NC, NTRI, 128, 13)[:, :, :119]
    return np.concatenate([rA, rB], axis=2)


def _device_model_numpy(qt8, ka8, kb8, vpa_h, vpb_h, adjm_h):
    """Numpy mirror of the device program (per core), on packed arrays."""
    f32 = np.float32
    rawA = np.zeros((NBIG, 128, TPB * 13), f32)
    rawB = np.zeros((NBIG, 128, TPB * 13), f32)
    poA = poB = None
    adjm = adjm_h.astype(f32)                  # (128, 4, 247)
    for u in range(NU):
        ps = np.zeros((128, 4, 247), f32)
        for hh in range(2):
            t = 2 * u + hh
            sg, j = divmod(t, 3)
            ti, s = divmod(sg, SPT)
            q = qt8[ti, 32 * j:32 * j + KP, :, s, :247].astype(f32)  # (6,2,247)
            ka = ka8[ti, 32 * j:32 * j + KP, :, s, :].astype(f32)    # (6,2,128)
            kb = kb8[ti, 32 * j:32 * j + KP, :, s, :].astype(f32)
            sA = np.zeros((128, 247), f32)
            sB = np.zeros((128, 247), f32)
            for jj in range(KT):
                sA += ka[:, jj].T @ q[:, jj]
                sB += kb[:, jj].T @ q[:, jj]
            ps[:, 2 * hh] = sA
            ps[:, 2 * hh + 1] = sB
        e = np.exp(SCALE * ps).astype(BF16).astype(f32)
        em = (e * adjm).astype(BF16).astype(f32)
        if u % 4 == 0:
            poA = np.zeros((128, 104), f32)
            poB = np.zeros((128, 104), f32)
        for hh in range(2):
            t = 2 * u + hh
            pcol = 13 * (t % 8)
            va = vpa_h[:, t, :].astype(f32)
            vb = vpb_h[:, t, :].astype(f32)
            emA, emB = em[:, 2 * hh], em[:, 2 * hh + 1]
            poA[:, pcol:pcol + 13] = (emA[:, :128].T @ va
                                      + emB[:, :128].T @ vb)
            poB[:119, pcol:pcol + 13] = (emA[:, 128:247].T @ va
                                         + emB[:, 128:247].T @ vb)
        if u % 4 == 3:
            t = 2 * u
            big, og = t // TPB, (t % TPB) // 8
            rawA[big, :, 104 * og:104 * og + 104] = poA
            rawB[big, :, 104 * og:104 * og + 104] = poB
    return rawA, rawB


def _prune_redundant_waits(nc):
    """Drop sync waits transitively implied by another wait on the same
    instruction: if wait w2's producing instruction itself waited on
    semaphore w.sem >= w.value, then w2 being satisfied implies w is too
    (sem values are monotonic).  Needed because walrus's HW-decoded PE
    matmul struct has a single sync-wait slot, and the tile scheduler
    emits (ACT, PE) wait pairs on PSUM-reuse matmuls where the PE wait
    is implied by the ACT one."""
    from collections import defaultdict
    for fn in nc.m.functions:
        for blk in fn.blocks:
            insts = list(blk.instructions)
            prod = defaultdict(list)
            for inst in insts:
                si = inst.sync_info
                if si is None:
                    continue
                for up in (si.on_update or []):
                    cum = (prod[up.ant_name][-1][0] if prod[up.ant_name]
                           else 0) + (up.update_value or 1)
                    prod[up.ant_name].append((cum, inst))

            def covers(w2, w):
                # True if waiting on w2 guarantees w is already satisfied.
                for cum, p in prod.get(w2.ant_name, []):
                    if cum >= w2.wait_value:
                        psi = p.sync_info
                        for pw in (psi.on_wait or []) if psi else []:
                            if (pw.ant_name == w.ant_name
                                    and pw.wait_value >= w.wait_value):
                                return True
                        return False
                return False

            for inst in insts:
                si = inst.sync_info
                if si is None or not si.on_wait or len(si.on_wait) < 2:
                    continue
                keep = list(si.on_wait)
                changed = True
                while changed and len(keep) > 1:
                    changed = False
                    for w in list(keep):
                        others = [x for x in keep if x is not w]
                        if any(covers(w2, w) for w2 in others):
                            keep.remove(w)
                            changed = True
                            break
                if len(keep) < len(si.on_wait):
                    si.on_wait = keep


def _build_device():
    import concourse.bass as bass
    import concourse.mybir as mybir
    from concourse import bacc, tile

    f32 = mybir.dt.float32
    bf = mybir.dt.bfloat16
    f8 = mybir.dt.float8e4
    DR = mybir.MatmulPerfMode.DoubleRow
    nc = bacc.Bacc("TRN2", target_bir_lowering=False)
    qt_d = nc.declare_dram_parameter("qt", [NQKT, 96, KT, SPT, 256], f8, isOutput=False)
    ka_d = nc.declare_dram_parameter("ka", [NQKT, 96, KT, SPT, 128], f8, isOutput=False)
    kb_d = nc.declare_dram_parameter("kb", [NQKT, 96, KT, SPT, 128], f8, isOutput=False)
    vpa_d = nc.declare_dram_parameter("vpa", [128, NTRI, 13], bf, isOutput=False)
    vpb_d = nc.declare_dram_parameter("vpb", [128, NTRI, 13], bf, isOutput=False)
    adjm_d = nc.declare_dram_parameter("adjm", [128, 4, 247], bf, isOutput=False)
    rawA_d = nc.declare_dram_parameter("rawA", [NBIG, 128, TPB * 13], f32, isOutput=True)
    rawB_d = nc.declare_dram_parameter("rawB", [NBIG, 128, TPB * 13], f32, isOutput=True)

    EXP = mybir.ActivationFunctionType.Exp
    with tile.TileContext(nc) as tc:
        with (
            tc.tile_pool(name="const", bufs=1) as cpool,
            tc.tile_pool(name="work", bufs=3) as work,
            tc.tile_pool(name="ostage", bufs=2) as opool,
            tc.tile_pool(name="psums", bufs=3, space=bass.MemorySpace.PSUM) as pps,
            tc.tile_pool(name="psumo", bufs=2, space=bass.MemorySpace.PSUM) as ppo,
        ):
            # consts first: unit 0's mask-mul and matmul2 block on them;
            # the bulk q/k tiles (needed progressively) follow.
            vpa_t = cpool.tile([128, NTRI, 13], bf, tag="vpa")
            nc.sync.dma_start(out=vpa_t[:], in_=vpa_d[:, :, :])
            vpb_t = cpool.tile([128, NTRI, 13], bf, tag="vpb")
            nc.sync.dma_start(out=vpb_t[:], in_=vpb_d[:, :, :])
            adjm_t = cpool.tile([128, 4, 247], bf, tag="adjm")
            nc.sync.dma_start(out=adjm_t[:], in_=adjm_d[:, :, :])
            qt_tiles, ka_tiles, kb_tiles = {}, {}, {}
            for ti in range(NQKT):
                qs_ = cpool.tile([96, KT, SPT, 256], f8, tag=f"qt{ti}")
                nc.sync.dma_start(out=qs_[:], in_=qt_d[ti])
                ks_ = cpool.tile([96, KT, SPT, 128], f8, tag=f"ka{ti}")
                nc.sync.dma_start(out=ks_[:], in_=ka_d[ti])
                kbs_ = cpool.tile([96, KT, SPT, 128], f8, tag=f"kb{ti}")
                nc.sync.dma_start(out=kbs_[:], in_=kb_d[ti])
                qt_tiles[ti], ka_tiles[ti], kb_tiles[ti] = qs_, ks_, kbs_

            ems = {}
            po = oA = oB = None
            for u in range(NU + 2):
                if u < NU:
                    ps = pps.tile([128, 4, 256], f32, tag="ps")
                    for hh in range(2):
                        t = 2 * u + hh
                        sg, j = divmod(t, 3)
                        ti, s = divmod(sg, SPT)
                        qs = qt_tiles[ti][32 * j:32 * j + KP, :, s, 0:247]
                        ka = ka_tiles[ti][32 * j:32 * j + KP, :, s, 0:128]
                        kb = kb_tiles[ti][32 * j:32 * j + KP, :, s, 0:128]
                        nc.tensor.matmul(ps[:, 2 * hh, 0:247], ka, qs,
                                         start=True, stop=True, perf_mode=DR)
                        nc.tensor.matmul(ps[:, 2 * hh + 1, 0:247], kb, qs,
                                         start=True, stop=True, perf_mode=DR)
                    e = work.tile([128, 4, 247], bf, tag="e")
                    nc.scalar.activation(e[:, :, :], ps[:, :, 0:247], EXP,
                                         scale=SCALE)
                    em = work.tile([128, 4, 247], bf, tag="em")
                    nc.vector.tensor_mul(em[:, :, :], e[:, :, :],
                                         adjm_t[:, :, :])
                    ems[u] = em
                v = u - 2
                if v >= 0:
                    em = ems.pop(v)
                    if v % 4 == 0:
                        po = ppo.tile([128, 512], f32, tag="po")
                    if v % (TPB // 2) == 0:
                        oA = opool.tile([128, TPB * 13], f32, tag="oA")
                        oB = opool.tile([128, TPB * 13], f32, tag="oB")
                    for hh in range(2):
                        t = 2 * v + hh
                        pcol = 13 * (t % 8)
                        va = vpa_t[:, t, :]
                        vb = vpb_t[:, t, :]
                        nc.tensor.matmul(po[:, pcol:pcol + 13],
                                         em[:, 2 * hh, 0:128], va,
                                         start=True, stop=False)
                        nc.tensor.matmul(po[:, pcol:pcol + 13],
                                         em[:, 2 * hh + 1, 0:128], vb,
                                         start=False, stop=True)
                        nc.tensor.matmul(po[:119, 256 + pcol:256 + pcol + 13],
                                         em[:, 2 * hh, 128:247], va,
                                         start=True, stop=False)
                        nc.tensor.matmul(po[:119, 256 + pcol:256 + pcol + 13],
                                         em[:, 2 * hh + 1, 128:247], vb,
                                         start=False, stop=True)
                    if v % 4 == 3:
                        t = 2 * v
                        big, og = t // TPB, (t % TPB) // 8
                        nc.vector.tensor_copy(oA[:, 104 * og:104 * og + 104],
                                              po[:, :104])
                        nc.vector.tensor_copy(oB[:119, 104 * og:104 * og + 104],
                                              po[:119, 256:360])
                        if og == 3:
                            nc.sync.dma_start(out=rawA_d[big], in_=oA[:])
                            nc.sync.dma_start(out=rawB_d[big], in_=oB[:])
    _prune_redundant_waits(nc)
    nc.finalize()
    return nc


def _attention_device(qt8, ka8, kb8, vpa_h, vpb_h, adjm_h):
    global LAST_EXEC_NS
    from concourse.bass_utils import run_bass_kernel_spmd
    if "nc" not in _DEVICE_CACHE:
        _DEVICE_CACHE["nc"] = _build_device()
    nc = _DEVICE_CACHE["nc"]
    in_maps = []
    for c in range(NCORES):
        in_maps.append({"qt": qt8[c], "ka": ka8[c], "kb": kb8[c],
                        "vpa": vpa_h[c], "vpb": vpb_h[c], "adjm": adjm_h})
    trace = bool(os.environ.get("KERNEL_TRACE"))
    res = run_bass_kernel_spmd(nc, in_maps, list(range(NCORES)), trace=trace)
    if trace:
        LAST_EXEC_NS = res.exec_time_ns
        _DEVICE_CACHE["last_results"] = res
    rawA = np.stack([res.results[c]["rawA"] for c in range(NCORES)])
    rawB = np.stack([res.results[c]["rawB"] for c in range(NCORES)])
    return rawA, rawB


def kernel(x, prc, adj, Wq, Wk, Wv, ln_g, ln_b, ln2_g, ln2_b, enc_w, enc_b,
           dec_w, dec_b, map_w, map_b, conv_w, conv_b, g1_w, g1_b,
           gln_g, gln_b, g2_w, g2_b):
    x = np.asarray(x, np.float32)
    prc = np.asarray(prc, np.float32)
    tr, ti = _front(x, map_w, map_b, conv_w, conv_b)

    # QKV projections for both re/im streams: t (B,N,L) @ W[h] (L,L)
    ts = np.stack([tr, ti], axis=1)                    # (B, 2, N, L)
    Q = np.einsum('brnl,hlo->brhon', ts, Wq)           # (B,2,H,L,N) = Q^T
    K = np.einsum('brnl,hlo->brhon', ts, Wk)
    V = np.einsum('brnl,hlo->brhno', ts, Wv)           # (B,2,H,N,L)
    ones = np.ones((B, 2, H, N, 1), np.float32)
    Vp = np.concatenate([V, ones], axis=-1)            # (B,2,H,N,L+1)

    qt = Q.reshape(NCORES, NTRI, L, N).astype(np.float32)
    kt = K.reshape(NCORES, NTRI, L, N).astype(np.float32)
    vp = Vp.reshape(NCORES, NTRI, N, L + 1).astype(np.float32)
    qt8, ka8, kb8, vpa_h, vpb_h, adjm_h = _pack_host(qt, kt, vp, adj)

    if os.environ.get("KERNEL_NUMPY"):
        outs = [_device_model_numpy(qt8[c], ka8[c], kb8[c], vpa_h[c],
                                    vpb_h[c], adjm_h) for c in range(NCORES)]
        rawA = np.stack([o[0] for o in outs])
        rawB = np.stack([o[1] for o in outs])
    else:
        try:
            rawA, rawB = _attention_device(qt8, ka8, kb8, vpa_h, vpb_h, adjm_h)
        except Exception as e:
            import traceback
            traceback.print_exc()
            print(f"DEVICE PATH FAILED ({e}); falling back to numpy")
            outs = [_device_model_numpy(qt8[c], ka8[c], kb8[c], vpa_h[c],
                                        vpb_h[c], adjm_h)
                    for c in range(NCORES)]
            rawA = np.stack([o[0] for o in outs])
            rawB = np.stack([o[1] for o in outs])

    raw = _unpack_raw(rawA, rawB)                      # (NC, NTRI, N, 13)
    raw = raw.reshape(B, 2, H, N, L + 1)
    out_av = raw[..., :L] / raw[..., L:L + 1]          # (B,2,H,N,L)
    out_ln = _ln(out_av, ln_g, ln_b)                   # post-attention LN

    res = []
    for ri in range(2):
        out = np.transpose(out_ln[:, ri], (1, 0, 2, 3))  # (H,B,N,L)
        hs = np.transpose(out, (1, 0, 2, 3))             # (B,H,N,L)
        nf = np.moveaxis(hs, 2, 0)                       # (N,B,H,L)
        nfr = nf.reshape(N, H, B, L)
        avg = nfr.mean(axis=1)                           # (N,B,L)
        mx = nfr.max(axis=1)
        z = np.concatenate([avg, mx], axis=-1) @ g1_w.T + g1_b
        z = _ln(z, gln_g, gln_b)
        z = z * 0.5 * (1.0 + _erf(z / math.sqrt(2.0)))   # exact gelu
        z = 1.0 / (1.0 + np.exp(-(z @ g2_w.T + g2_b)))
        fused = z * avg + (1.0 - z) * mx + nf.mean(axis=2)
        res.append(np.transpose(fused, (1, 0, 2)))       # (B,N,L)

    xr = _softshrink(res[0])
    xi = _softshrink(res[1])
    f = xr * enc_w[0, 0] + xi * enc_w[0, 1] + enc_b[0]
    f = _ln(f, ln2_g, ln2_b) + x
    e = f * enc_w[0, 0] + prc * enc_w[0, 1] + enc_b[0]
    a = _leaky(e)
    out = a @ dec_w.T + dec_b + x
    return out.astype(np.float32)


# revision 9
# speedup vs baseline: 1.0169x; 1.0169x over previous
"""nn_CoupFourGAT — Trainium2 Bass kernel.

Host (numpy): STFT framing + rFFT + map_w projection + 3x3 conv + QKV
projections + gate-fusion MLP + decoder (all tiny, ~100 MFLOP total).
Device (8 NeuronCores, SPMD): the dominant masked-softmax GAT attention:
per (batch, re/im, head) triple computes
  E^T = adj^T * exp(scale * K Q^T);  raw = [E @ V | E @ 1]
with the softmax denominator fused in as an extra ones-column of V.
Data-parallel over batch: core c handles batches [16c, 16c+16).

v3 design (ACT-saturated pipeline):
 - MM1 (K Q^T) bf16, tight 247 moving cols per (triple, m-chunk);
   3 triples per 32-partition group (bases {0,32,64}).
 - scores psum tile [128, 4, 256] f32 (2 banks) holds 2 triples x 2
   m-chunks (m 0:128 / 119:247, 247 query cols each).
 - one exp ACT per pair ([128, 4x247] strided AP, scale folded in,
   bf16 out) -> one DVE adj-mask mul (bf16 2x mode).
 - MM2 (E^T.T @ Vp, em-stationary) accumulates 8 triples per PSUM
   bank; DVE drains PSUM->SBUF; 8 output DMAs.
 - psum: scores pool bufs=3 (6 banks) + po pool bufs=2 (2 banks);
   MM1 emitted 2 iterations ahead of MM2 so the in-order PE queue
   never blocks the next ACT.
 - q/k DMAs carry only the 18 used partitions (3 DMAs per tile at
   bases {0,32,64}); tile-0 q/k ships first so MM1(0) starts ~7.5us.
"""
import math
import os
import numpy as np
import ml_dtypes

B, N, L, H = 128, 247, 12, 4
NFFT, HOP, FRAMES = 256, 246, 13
LAM = 0.01
NCORES = 8
BLOC = B // NCORES          # 16 batches per core
NTRI = BLOC * 2 * H         # 128 (batch, re/im, head) triples per core
NBIG = 4                    # output big groups per core
TPB = NTRI // NBIG          # 32 triples per output big group
NSLOT = (NTRI + 2) // 3     # 43 slots of 3 triples (bases {0,32,64})
SPT = 8                     # slots per qt/ka/kb tile
NQKT = (NSLOT + SPT - 1) // SPT  # 6 q/k tiles
NU = NTRI // 2              # 64 u-iterations (2 triples each)
SCALE = 1.0 / math.sqrt(L)
BF16 = ml_dtypes.bfloat16

_DEVICE_CACHE = {}
LAST_EXEC_NS = None


def _erf(x):
    try:
        from scipy.special import erf
        return erf(x)
    except Exception:
        return np.vectorize(math.erf, otypes=[np.float64])(x)


def _ln(t, g, b, eps=1e-5):
    m = t.mean(-1, keepdims=True)
    v = ((t - m) ** 2).mean(-1, keepdims=True)
    return (t - m) / np.sqrt(v + eps) * g + b


def _softshrink(t):
    return np.where(t > LAM, t - LAM, np.where(t < -LAM, t + LAM, 0.0))


def _leaky(t):
    return np.where(t >= 0, t, 0.01 * t)


def _front(x, map_w, map_b, conv_w, conv_b):
    """x (B,N,L) -> tr, ti (B,N,L): FFT + map + conv + leaky + residual."""
    Bc = x.shape[0]
    xf = x.reshape(Bc, -1)
    pad = np.pad(xf, ((0, 0), (NFFT // 2, NFFT // 2)), mode='reflect')
    idx = np.arange(FRAMES)[:, None] * HOP + np.arange(NFFT)[None, :]
    frames = pad[:, idx]                               # (B, 13, 256)
    spec = np.fft.rfft(frames.astype(np.float64), axis=-1)  # (B,13,129)
    spec = np.swapaxes(spec, 1, 2)[:, :, :L]           # (B, 129, 12)
    xr = spec.real.reshape(Bc, L, 129) @ map_w.T.astype(np.float64) + map_b
    xi = spec.imag.reshape(Bc, L, 129) @ map_w.T.astype(np.float64) + map_b
    xr = xr.reshape(Bc, N, L).astype(np.float32)
    xi = xi.reshape(Bc, N, L).astype(np.float32)
    vec = np.stack([xr, xi], axis=-1)                  # (B, N, L, 2)
    v2 = vec.reshape(Bc, 2, N, L)
    vp = np.pad(v2, ((0, 0), (0, 0), (1, 1), (1, 1)))
    c = np.zeros_like(v2)
    for o in range(2):
        for i in range(2):
            for ky in range(3):
                for kx in range(3):
                    c[:, o] += conv_w[o, i, ky, kx] * vp[:, i, ky:ky + N, kx:kx + L]
    c = c + conv_b[None, :, None, None]
    c = _leaky(c.reshape(Bc, N, L, 2)) + vec
    return c[..., 0], c[..., 1]


def _pack_host(qt, kt, vp, adj):
    """qt/kt: (NC, NTRI, L, N) f32; vp: (NC, NTRI, N, 13) f32; adj (N, N).

    Returns per-core device arrays:
      qt8 (NC, NQKT, 3, L, SPT, 256) bf16   [only 18 used partition rows]
      ka8/kb8 (NC, NQKT, 3, L, SPT, 128)    [key rows m 0:128 / 119:247]
      vpa/vpb (NC, 128, NTRI, 13) bf16      [m-chunks, overlap zeroed in B]
      adjm (128, 4, 247) bf16               [mask blocks A,B,A,B]
    """
    NC = NCORES

    def qk_pack(a, width, mlo):
        # a: (NC, NTRI, L, >=mlo+width):
        # out[c, ti, j, l, s, w] = a[c, t=3(8ti+s)+j, l, mlo+w]
        pad = np.zeros((NC, NQKT * SPT * 3, L, width), a.dtype)
        pad[:, :NTRI] = a[:, :, :, mlo:mlo + width]
        src = pad.reshape(NC, NQKT, SPT, 3, L, width)
        return np.ascontiguousarray(
            src.transpose(0, 1, 3, 4, 2, 5).astype(BF16))

    qn = np.zeros((NC, NTRI, L, 256), np.float32)
    qn[:, :, :, :N] = qt
    qt8 = qk_pack(qn, 256, 0)
    ka8 = qk_pack(kt, 128, 0)
    kb8 = qk_pack(kt, 128, 119)

    vpa = vp[:, :, :128, :]                                  # (NC,NTRI,128,13)
    vpb = vp[:, :, 119:, :].copy()
    vpb[:, :, :9, :] = 0.0                                   # kill overlap
    vpa_h = np.ascontiguousarray(vpa.transpose(0, 2, 1, 3)).astype(BF16)
    vpb_h = np.ascontiguousarray(vpb.transpose(0, 2, 1, 3)).astype(BF16)

    adjT = adj.T.astype(np.float32)                          # adjT[m,n]
    mA = adjT[:128, :]                                       # (128, 247)
    mB = adjT[119:, :]
    adjm_h = np.ascontiguousarray(
        np.stack([mA, mB, mA, mB], axis=1)).astype(BF16)     # (128, 4, 247)
    return qt8, ka8, kb8, vpa_h, vpb_h, adjm_h


def _unpack_raw(rawA, rawB):
    """rawA/rawB (NC, NBIG, 128, 416) f32 -> raw (NC, NTRI, N, 13)."""
    NC = NCORES
    rA = rawA.reshape(NC, NBIG, 128, TPB, 13).transpose(0, 1, 3, 2, 4)
    rA = rA.reshape(NC, NTRI, 128, 13)
    rB = rawB.reshape(NC, NBIG, 128, TPB, 13).transpose(0, 1, 3, 2, 4)
    rB = rB.reshape(NC, NTRI, 128, 13)[:, :, :119]
    return np.concatenate([rA, rB], axis=2)


def _device_model_numpy(qt8, ka8, kb8, vpa_h, vpb_h, adjm_h):
    """Numpy mirror of the device program (per core), on packed arrays."""
    f32 = np.float32
    rawA = np.zeros((NBIG, 128, TPB * 13), f32)
    rawB = np.zeros((NBIG, 128, TPB * 13), f32)
    poA = poB = None
    adjm = adjm_h.astype(f32)                  # (128, 4, 247)
    for u in range(NU):
        ps = np.zeros((128, 4, 247), f32)
        for hh in range(2):
            t = 2 * u + hh
            sg, j = divmod(t, 3)
            ti, s = divmod(sg, SPT)
            q = qt8[ti, j, :, s, :247].astype(f32)     # (12, 247)
            ka = ka8[ti, j, :, s, :].astype(f32)       # (12, 128)
            kb = kb8[ti, j, :, s, :].astype(f32)
            ps[:, 2 * hh] = ka.T @ q
            ps[:, 2 * hh + 1] = kb.T @ q
        e = np.exp(SCALE * ps).astype(BF16).astype(f32)
        em = (e * adjm).astype(BF16).astype(f32)
        if u % 4 == 0:
            poA = np.zeros((128, 104), f32)
            poB = np.zeros((128, 104), f32)
        for hh in range(2):
            t = 2 * u + hh
            pcol = 13 * (t % 8)
            va = vpa_h[:, t, :].astype(f32)
            vb = vpb_h[:, t, :].astype(f32)
            emA, emB = em[:, 2 * hh], em[:, 2 * hh + 1]
            poA[:, pcol:pcol + 13] = (emA[:, :128].T @ va
                                      + emB[:, :128].T @ vb)
            poB[:119, pcol:pcol + 13] = (emA[:, 128:247].T @ va
                                         + emB[:, 128:247].T @ vb)
        if u % 4 == 3:
            t = 2 * u
            big, og = t // TPB, (t % TPB) // 8
            rawA[big, :, 104 * og:104 * og + 104] = poA
            rawB[big, :, 104 * og:104 * og + 104] = poB
    return rawA, rawB


def _prune_redundant_waits(nc):
    """Drop sync waits transitively implied by another wait on the same
    instruction: if wait w2's producing instruction itself waited on
    semaphore w.sem >= w.value, then w2 being satisfied implies w is too
    (sem values are monotonic).  Needed because walrus's HW-decoded PE
    matmul struct has a single sync-wait slot, and the tile scheduler
    emits (ACT, PE) wait pairs on PSUM-reuse matmuls where the PE wait
    is implied by the ACT one."""
    from collections import defaultdict
    for fn in nc.m.functions:
        for blk in fn.blocks:
            insts = list(blk.instructions)
            prod = defaultdict(list)
            for inst in insts:
                si = inst.sync_info
                if si is None:
                    continue
                for up in (si.on_update or []):
                    cum = (prod[up.ant_name][-1][0] if prod[up.ant_name]
                           else 0) + (up.update_value or 1)
                    prod[up.ant_name].append((cum, inst))

            def covers(w2, w):
                # True if waiting on w2 guarantees w is already satisfied.
                for cum, p in prod.get(w2.ant_name, []):
                    if cum >= w2.wait_value:
                        psi = p.sync_info
                        for pw in (psi.on_wait or []) if psi else []:
                            if (pw.ant_name == w.ant_name
                                    and pw.wait_value >= w.wait_value):
                                return True
                        return False
                return False

            for inst in insts:
                si = inst.sync_info
                if si is None or not si.on_wait or len(si.on_wait) < 2:
                    continue
                keep = list(si.on_wait)
                changed = True
                while changed and len(keep) > 1:
                    changed = False
                    for w in list(keep):
                        others = [x for x in keep if x is not w]
                        if any(covers(w2, w) for w2 in others):
                            keep.remove(w)
                            changed = True
                            break
                if len(keep) < len(si.on_wait):
                    si.on_wait = keep


def _build_device():
    import concourse.bass as bass
    import concourse.mybir as mybir
    from concourse import bacc, tile

    f32 = mybir.dt.float32
    bf = mybir.dt.bfloat16
    nc = bacc.Bacc("TRN2", target_bir_lowering=False)
    qt_d = nc.declare_dram_parameter("qt", [NQKT, 3, L, SPT, 256], bf, isOutput=False)
    ka_d = nc.declare_dram_parameter("ka", [NQKT, 3, L, SPT, 128], bf, isOutput=False)
    kb_d = nc.declare_dram_parameter("kb", [NQKT, 3, L, SPT, 128], bf, isOutput=False)
    vpa_d = nc.declare_dram_parameter("vpa", [128, NTRI, 13], bf, isOutput=False)
    vpb_d = nc.declare_dram_parameter("vpb", [128, NTRI, 13], bf, isOutput=False)
    adjm_d = nc.declare_dram_parameter("adjm", [128, 4, 247], bf, isOutput=False)
    rawA_d = nc.declare_dram_parameter("rawA", [NBIG, 128, TPB * 13], f32, isOutput=True)
    rawB_d = nc.declare_dram_parameter("rawB", [NBIG, 128, TPB * 13], f32, isOutput=True)

    EXP = mybir.ActivationFunctionType.Exp
    with tile.TileContext(nc) as tc:
        with (
            tc.tile_pool(name="const", bufs=1) as cpool,
            tc.tile_pool(name="work", bufs=3) as work,
            tc.tile_pool(name="ostage", bufs=2) as opool,
            tc.tile_pool(name="psums", bufs=3, space=bass.MemorySpace.PSUM) as pps,
            tc.tile_pool(name="psumo", bufs=2, space=bass.MemorySpace.PSUM) as ppo,
        ):
            # DMA order: tile-0 q/k first (first MM1 blocks on it), then
            # the consts (mask for DVE(0), Vp for MM2(0)), then the rest.
            qt_tiles, ka_tiles, kb_tiles = {}, {}, {}

            def qk_dma(ti):
                qs_ = cpool.tile([96, SPT, 256], bf, tag=f"qt{ti}")
                ks_ = cpool.tile([96, SPT, 128], bf, tag=f"ka{ti}")
                kbs_ = cpool.tile([96, SPT, 128], bf, tag=f"kb{ti}")
                for j in range(3):
                    nc.sync.dma_start(out=qs_[32 * j:32 * j + L, :, :],
                                      in_=qt_d[ti, j])
                    nc.sync.dma_start(out=ks_[32 * j:32 * j + L, :, :],
                                      in_=ka_d[ti, j])
                    nc.sync.dma_start(out=kbs_[32 * j:32 * j + L, :, :],
                                      in_=kb_d[ti, j])
                qt_tiles[ti], ka_tiles[ti], kb_tiles[ti] = qs_, ks_, kbs_

            qk_dma(0)
            adjm_t = cpool.tile([128, 4, 247], bf, tag="adjm")
            nc.sync.dma_start(out=adjm_t[:], in_=adjm_d[:, :, :])
            vpa_t = cpool.tile([128, NTRI, 13], bf, tag="vpa")
            nc.sync.dma_start(out=vpa_t[:], in_=vpa_d[:, :, :])
            vpb_t = cpool.tile([128, NTRI, 13], bf, tag="vpb")
            nc.sync.dma_start(out=vpb_t[:], in_=vpb_d[:, :, :])
            for ti in range(1, NQKT):
                qk_dma(ti)

            ems = {}
            po = oA = oB = None
            for u in range(NU + 2):
                if u < NU:
                    ps = pps.tile([128, 4, 256], f32, tag="ps")
                    for hh in range(2):
                        t = 2 * u + hh
                        sg, j = divmod(t, 3)
                        ti, s = divmod(sg, SPT)
                        qs = qt_tiles[ti][32 * j:32 * j + L, s, 0:247]
                        ka = ka_tiles[ti][32 * j:32 * j + L, s, 0:128]
                        kb = kb_tiles[ti][32 * j:32 * j + L, s, 0:128]
                        nc.tensor.matmul(ps[:, 2 * hh, 0:247], ka, qs,
                                         start=True, stop=True)
                        nc.tensor.matmul(ps[:, 2 * hh + 1, 0:247], kb, qs,
                                         start=True, stop=True)
                    e = work.tile([128, 4, 256], bf, tag="e")
                    nc.scalar.activation(e[:, :, 0:247], ps[:, :, 0:247], EXP,
                                         scale=SCALE)
                    em = work.tile([128, 4, 256], bf, tag="em")
                    nc.vector.tensor_mul(em[:, :, 0:247], e[:, :, 0:247],
                                         adjm_t[:, :, :])
                    ems[u] = em
                v = u - 2
                if v >= 0:
                    em = ems.pop(v)
                    if v % 4 == 0:
                        po = ppo.tile([128, 512], f32, tag="po")
                    if v % (TPB // 2) == 0:
                        oA = opool.tile([128, TPB * 13], f32, tag="oA")
                        oB = opool.tile([128, TPB * 13], f32, tag="oB")
                    for hh in range(2):
                        t = 2 * v + hh
                        pcol = 13 * (t % 8)
                        va = vpa_t[:, t, :]
                        vb = vpb_t[:, t, :]
                        nc.tensor.matmul(po[:, pcol:pcol + 13],
                                         em[:, 2 * hh, 0:128], va,
                                         start=True, stop=False)
                        nc.tensor.matmul(po[:, pcol:pcol + 13],
                                         em[:, 2 * hh + 1, 0:128], vb,
                                         start=False, stop=True)
                        nc.tensor.matmul(po[:119, 256 + pcol:256 + pcol + 13],
                                         em[:, 2 * hh, 128:247], va,
                                         start=True, stop=False)
                        nc.tensor.matmul(po[:119, 256 + pcol:256 + pcol + 13],
                                         em[:, 2 * hh + 1, 128:247], vb,
                                         start=False, stop=True)
                    if v % 4 == 3:
                        t = 2 * v
                        big, og = t // TPB, (t % TPB) // 8
                        nc.vector.tensor_copy(oA[:, 104 * og:104 * og + 104],
                                              po[:, :104])
                        nc.vector.tensor_copy(oB[:119, 104 * og:104 * og + 104],
                                              po[:119, 256:360])
                        if og == 3:
                            nc.sync.dma_start(out=rawA_d[big], in_=oA[:])
                            nc.sync.dma_start(out=rawB_d[big], in_=oB[:])
    _prune_redundant_waits(nc)
    nc.finalize()
    return nc


def _attention_device(qt8, ka8, kb8, vpa_h, vpb_h, adjm_h):
    global LAST_EXEC_NS
    from concourse.bass_utils import run_bass_kernel_spmd
    if "nc" not in _DEVICE_CACHE:
        _DEVICE_CACHE["nc"] = _build_device()
    nc = _DEVICE_CACHE["nc"]
    in_maps = []
    for c in range(NCORES):
        in_maps.append({"qt": qt8[c], "ka": ka8[c], "kb": kb8[c],
                        "vpa": vpa_h[c], "vpb": vpb_h[c], "adjm": adjm_h})
    trace = bool(os.environ.get("KERNEL_TRACE"))
    res = run_bass_kernel_spmd(nc, in_maps, list(range(NCORES)), trace=trace)
    if trace:
        LAST_EXEC_NS = res.exec_time_ns
        _DEVICE_CACHE["last_results"] = res
    rawA = np.stack([res.results[c]["rawA"] for c in range(NCORES)])
    rawB = np.stack([res.results[c]["rawB"] for c in range(NCORES)])
    return rawA, rawB


def kernel(x, prc, adj, Wq, Wk, Wv, ln_g, ln_b, ln2_g, ln2_b, enc_w, enc_b,
           dec_w, dec_b, map_w, map_b, conv_w, conv_b, g1_w, g1_b,
           gln_g, gln_b, g2_w, g2_b):
    x = np.asarray(x, np.float32)
    prc = np.asarray(prc, np.float32)
    tr, ti = _front(x, map_w, map_b, conv_w, conv_b)

    # QKV projections for both re/im streams: t (B,N,L) @ W[h] (L,L)
    ts = np.stack([tr, ti], axis=1)                    # (B, 2, N, L)
    Q = np.einsum('brnl,hlo->brhon', ts, Wq)           # (B,2,H,L,N) = Q^T
    K = np.einsum('brnl,hlo->brhon', ts, Wk)
    V = np.einsum('brnl,hlo->brhno', ts, Wv)           # (B,2,H,N,L)
    ones = np.ones((B, 2, H, N, 1), np.float32)
    Vp = np.concatenate([V, ones], axis=-1)            # (B,2,H,N,L+1)

    qt = Q.reshape(NCORES, NTRI, L, N).astype(np.float32)
    kt = K.reshape(NCORES, NTRI, L, N).astype(np.float32)
    vp = Vp.reshape(NCORES, NTRI, N, L + 1).astype(np.float32)
    qt8, ka8, kb8, vpa_h, vpb_h, adjm_h = _pack_host(qt, kt, vp, adj)

    if os.environ.get("KERNEL_NUMPY"):
        outs = [_device_model_numpy(qt8[c], ka8[c], kb8[c], vpa_h[c],
                                    vpb_h[c], adjm_h) for c in range(NCORES)]
        rawA = np.stack([o[0] for o in outs])
        rawB = np.stack([o[1] for o in outs])
    else:
        try:
            rawA, rawB = _attention_device(qt8, ka8, kb8, vpa_h, vpb_h, adjm_h)
        except Exception as e:
            import traceback
            traceback.print_exc()
            print(f"DEVICE PATH FAILED ({e}); falling back to numpy")
            outs = [_device_model_numpy(qt8[c], ka8[c], kb8[c], vpa_h[c],
                                        vpb_h[c], adjm_h)
                    for c in range(NCORES)]
            rawA = np.stack([o[0] for o in outs])
            rawB = np.stack([o[1] for o in outs])

    raw = _unpack_raw(rawA, rawB)                      # (NC, NTRI, N, 13)
    raw = raw.reshape(B, 2, H, N, L + 1)
    out_av = raw[..., :L] / raw[..., L:L + 1]          # (B,2,H,N,L)
    out_ln = _ln(out_av, ln_g, ln_b)                   # post-attention LN

    res = []
    for ri in range(2):
        out = np.transpose(out_ln[:, ri], (1, 0, 2, 3))  # (H,B,N,L)
        hs = np.transpose(out, (1, 0, 2, 3))             # (B,H,N,L)
        nf = np.moveaxis(hs, 2, 0)                       # (N,B,H,L)
        nfr = nf.reshape(N, H, B, L)
        avg = nfr.mean(axis=1)                           # (N,B,L)
        mx = nfr.max(axis=1)
        z = np.concatenate([avg, mx], axis=-1) @ g1_w.T + g1_b
        z = _ln(z, gln_g, gln_b)
        z = z * 0.5 * (1.0 + _erf(z / math.sqrt(2.0)))   # exact gelu
        z = 1.0 / (1.0 + np.exp(-(z @ g2_w.T + g2_b)))
        fused = z * avg + (1.0 - z) * mx + nf.mean(axis=2)
        res.append(np.transpose(fused, (1, 0, 2)))       # (B,N,L)

    xr = _softshrink(res[0])
    xi = _softshrink(res[1])
    f = xr * enc_w[0, 0] + xi * enc_w[0, 1] + enc_b[0]
    f = _ln(f, ln2_g, ln2_b) + x
    e = f * enc_w[0, 0] + prc * enc_w[0, 1] + enc_b[0]
    a = _leaky(e)
    out = a @ dec_w.T + dec_b + x
    return out.astype(np.float32)
